# revision 6
# baseline (speedup 1.0000x reference)
"""Trainium2 Bass kernel for the SCAN-style cross-attention contrastive loss.

Sharding: image axis across 8 cores (8 images/core), captions replicated.
Each core computes its 66x8 column block of per-(caption,image) exp-sum
scores; the host gathers columns and applies the scalar hinge-loss epilogue.

Math restructure (validated to ~1e-7 against the jax reference):
  - unnormalized softmax weights u = exp(9*A_norm + wbias); the softmax
    denominator cancels in sim = num/(n1*||wctx||).
  - num  = E^T (u .* Araw)          (per-column reduction via indicator matmul)
  - q    = E^T (u .* (G_blk @ u)) = ||wctx_unnorm||^2 via per-caption Gram
  - invalid image frames are zeroed on host => their columns give e = 1
    exactly; host subtracts the known defect (F - img_len) from each exp-sum.
"""
from contextlib import ExitStack

import numpy as np

import concourse.bacc as bacc
import concourse.tile as tile
from concourse import mybir
from concourse.bass_utils import run_bass_kernel_spmd

N, F, W, D = 64, 64, 40, 512
NCORES = 8
IPC = N // NCORES        # images per core = 8
IF = IPC * F             # 512 image-frame columns per core
GP = 3                   # captions per partition group
NCAP = 66                # 64 captions padded to a multiple of GP
NG = NCAP // GP          # 22 groups
GW = GP * W              # 120 partitions per group
DCH = D // 128           # 4 contraction chunks

f32 = mybir.dt.float32
f32r = mybir.dt.float32r
FT = mybir.ActivationFunctionType
ALU = mybir.AluOpType
AX = mybir.AxisListType

MARGIN = 0.2
LAMBDA_LSE = 6.0


def _r(ap):
    return ap.bitcast(f32r)


def _build_nc():
    nc = bacc.Bacc("TRN2", target_bir_lowering=False, debug=False)
    capT = nc.dram_tensor("capT", [128, NG, DCH, GW], f32, kind="ExternalInput").ap()
    imgT = nc.dram_tensor("imgT", [128, DCH, IF], f32, kind="ExternalInput").ap()
    wbias = nc.dram_tensor("wbias", [GW, NG], f32, kind="ExternalInput").ap()
    gmask = nc.dram_tensor("gmask", [GW, GW], f32, kind="ExternalInput").ap()
    emat = nc.dram_tensor("emat", [GW, GP], f32, kind="ExternalInput").ap()
    ones = nc.dram_tensor("ones", [128, 1], f32, kind="ExternalInput").ap()
    se_out = nc.dram_tensor("se_out", [NCAP, IPC], f32, kind="ExternalOutput").ap()

    with tile.TileContext(nc) as tc, ExitStack() as ctx:
        const = ctx.enter_context(tc.tile_pool(name="const", bufs=1))
        caps = ctx.enter_context(tc.tile_pool(name="caps", bufs=3))
        work = ctx.enter_context(tc.tile_pool(name="work", bufs=2))
        small = ctx.enter_context(tc.tile_pool(name="small", bufs=3))
        pa = ctx.enter_context(tc.tile_pool(name="pa", bufs=2, space="PSUM"))
        pg = ctx.enter_context(tc.tile_pool(name="pg", bufs=2, space="PSUM"))
        pb = ctx.enter_context(tc.tile_pool(name="pb", bufs=1, space="PSUM"))
        pq = ctx.enter_context(tc.tile_pool(name="pq", bufs=1, space="PSUM"))

        imgT_t = const.tile([128, DCH, IF], f32r)
        nc.sync.dma_start(out=imgT_t, in_=imgT.bitcast(f32r))
        wbias_t = const.tile([GW, NG], f32)
        nc.sync.dma_start(out=wbias_t, in_=wbias)
        gmask_t = const.tile([GW, GW], f32)
        nc.sync.dma_start(out=gmask_t, in_=gmask)
        emat_t = const.tile([GW, GP], f32r)
        nc.sync.dma_start(out=emat_t, in_=emat.bitcast(f32r))
        ones_col = const.tile([128, 1], f32r)
        nc.sync.dma_start(out=ones_col, in_=ones.bitcast(f32r))
        eps_col = const.tile([128, 1], f32)
        nc.vector.memset(eps_col, 1e-20)

        # n1sq[f] = ||image frame f||^2, replicated across partitions
        imgsq_t = const.tile([128, DCH, IF], f32r)
        nc.vector.tensor_mul(imgsq_t, imgT_t.bitcast(f32), imgT_t.bitcast(f32))
        n1p = pb.tile([1, IF], f32, tag="n1")
        for c in range(DCH):
            nc.tensor.matmul(out=n1p, lhsT=ones_col, rhs=imgsq_t[:, c, :],
                             start=(c == 0), stop=(c == DCH - 1))
        n1row = const.tile([1, IF], f32)
        nc.scalar.copy(n1row, n1p)
        n1repl = const.tile([128, IF], f32)
        nc.gpsimd.partition_broadcast(n1repl, n1row[0:1, :])

        for g in range(NG):
            capg = caps.tile([128, DCH, GW], f32r)
            nc.sync.dma_start(out=capg, in_=capT[:, g, :, :].bitcast(f32r))

            # Araw[w, if] = caption_word . image_frame
            araw_p = pa.tile([GW, IF], f32)
            for c in range(DCH):
                nc.tensor.matmul(out=araw_p, lhsT=capg[:, c, :],
                                 rhs=imgT_t[:, c, :],
                                 start=(c == 0), stop=(c == DCH - 1))

            # per-caption Gram (block-diagonal after masking)
            gram_p = pg.tile([GW, GW], f32)
            for c in range(DCH):
                nc.tensor.matmul(out=gram_p, lhsT=capg[:, c, :],
                                 rhs=capg[:, c, :],
                                 start=(c == 0), stop=(c == DCH - 1))
            gblk_t = work.tile([GW, GW], f32r)
            nc.vector.tensor_mul(gblk_t, gram_p, gmask_t)

            araw_t = work.tile([GW, IF], f32)
            nc.scalar.copy(araw_t, araw_p)

            # leaky relu: L = max(0.1*A, A)
            L_t = work.tile([GW, IF], f32)
            nc.vector.scalar_tensor_tensor(out=L_t, in0=araw_t, scalar=0.1,
                                           in1=araw_t, op0=ALU.mult, op1=ALU.max)

            sq_t = work.tile([GW, IF], f32)
            nc.gpsimd.tensor_mul(sq_t, L_t, L_t)
            r2_t = small.tile([GW, IPC], f32)
            nc.vector.reduce_sum(r2_t, sq_t.rearrange("p (i f) -> p i f", f=F),
                                 axis=AX.X)
            sr_t = small.tile([GW, IPC], f32)
            nc.scalar.activation(sr_t, r2_t, FT.Sqrt)
            rinv_t = small.tile([GW, IPC], f32)
            nc.vector.reciprocal(rinv_t, sr_t)

            # A_norm = L * rinv (broadcast over frames), u = exp(9*A_norm + wbias)
            at_t = work.tile([GW, IF], f32)
            nc.vector.tensor_mul(at_t.rearrange("p (i f) -> p i f", f=F),
                                 L_t.rearrange("p (i f) -> p i f", f=F),
                                 rinv_t.to_broadcast([GW, IPC, F]))
            u_t = work.tile([GW, IF], f32r)
            nc.scalar.activation(u_t, at_t, FT.Exp, scale=9.0,
                                 bias=wbias_t[:, g:g + 1])

            b_p = pb.tile([GW, IF], f32, tag="b")
            nc.tensor.matmul(out=b_p, lhsT=gblk_t, rhs=u_t,
                             start=True, stop=True)

            p_t = work.tile([GW, IF], f32r)
            nc.vector.tensor_mul(p_t, u_t.bitcast(f32), b_p)
            q_t = work.tile([GW, IF], f32r)
            nc.gpsimd.tensor_mul(q_t, u_t.bitcast(f32), araw_t)

            qp = pq.tile([GP, IF], f32, tag="q")
            nc.tensor.matmul(out=qp, lhsT=emat_t, rhs=p_t,
                             start=True, stop=True)
            nump = pq.tile([GP, IF], f32, tag="num")
            nc.tensor.matmul(out=nump, lhsT=emat_t, rhs=q_t,
                             start=True, stop=True)

            # sim = num / sqrt(q * n1sq), e = exp(6*sim), block-sum over frames
            qs_t = work.tile([GP, IF], f32)
            nc.vector.tensor_mul(qs_t, qp, n1repl[0:GP, :])
            d_t = work.tile([GP, IF], f32)
            nc.scalar.activation(d_t, qs_t, FT.Sqrt, bias=eps_col[0:GP, :])
            ri2_t = work.tile([GP, IF], f32)
            nc.vector.reciprocal(ri2_t, d_t)
            sim_t = work.tile([GP, IF], f32)
            nc.vector.tensor_mul(sim_t, nump, ri2_t)
            e_t = work.tile([GP, IF], f32)
            nc.scalar.activation(e_t, sim_t, FT.Exp, scale=LAMBDA_LSE)
            seg_t = small.tile([GP, IPC], f32)
            nc.vector.reduce_sum(seg_t, e_t.rearrange("p (i f) -> p i f", f=F),
                                 axis=AX.X)
            nc.sync.dma_start(out=se_out[g * GP:(g + 1) * GP, :], in_=seg_t)

    nc.compile()
    return nc


_NC = None


def _get_nc():
    global _NC
    if _NC is None:
        _NC = _build_nc()
    return _NC


def make_in_maps(images, captions, img_lens, cap_lens):
    """Host-side input preparation (numpy only): shard/transpose/mask."""
    images = np.ascontiguousarray(np.asarray(images, np.float32))
    captions = np.ascontiguousarray(np.asarray(captions, np.float32))
    img_lens = np.asarray(img_lens).astype(np.int64)
    cap_lens = np.asarray(cap_lens).astype(np.int64)

    # captions padded to 66; dummies replicate caption 0 (avoids 0/0)
    caps_p = np.concatenate(
        [captions, np.broadcast_to(captions[0:1], (NCAP - N, W, D))], axis=0)
    # [128, NG, DCH, GW] with partition = d % 128, GW index = b*W + w
    capT_np = np.ascontiguousarray(
        caps_p.reshape(NG, GP, W, DCH, 128).transpose(4, 0, 3, 1, 2)
        .reshape(128, NG, DCH, GW))

    wbias_np = np.full((NCAP, W), np.float32(-1e30))
    for j in range(N):
        wbias_np[j, :cap_lens[j]] = 0.0
    wbias_np = np.ascontiguousarray(
        wbias_np.reshape(NG, GP * W).T.astype(np.float32))  # [GW, NG]

    gmask_np = np.zeros((GW, GW), np.float32)
    emat_np = np.zeros((GW, GP), np.float32)
    for b in range(GP):
        gmask_np[b * W:(b + 1) * W, b * W:(b + 1) * W] = 1.0
        emat_np[b * W:(b + 1) * W, b] = 1.0

    in_maps = []
    for core in range(NCORES):
        imgs = images[core * IPC:(core + 1) * IPC].copy()
        for i in range(IPC):
            imgs[i, img_lens[core * IPC + i]:] = 0.0
        Z = imgs.reshape(IF, D)
        imgT_np = np.ascontiguousarray(
            Z.reshape(IF, DCH, 128).transpose(2, 1, 0))  # [128, DCH, IF]
        in_maps.append({
            "capT": capT_np, "imgT": imgT_np, "wbias": wbias_np,
            "gmask": gmask_np, "emat": emat_np,
            "ones": np.ones((128, 1), np.float32),
        })
    return in_maps


def finish(se_list, img_lens):
    """Host epilogue: defect correction, log-sum-exp, hinge loss."""
    img_lens = np.asarray(img_lens).astype(np.int64)
    cols = []
    for core in range(NCORES):
        se = np.asarray(se_list[core], np.float32)[:N, :]         # (64, 8)
        defect = (F - img_lens[core * IPC:(core + 1) * IPC]).astype(np.float32)
        cols.append(np.log(se - defect[None, :]) / LAMBDA_LSE)
    S = np.concatenate(cols, axis=1).astype(np.float32)           # (caps, imgs)

    diag = np.diag(S)
    eye = np.eye(N, dtype=bool)
    cost_s = np.maximum(MARGIN + S - diag[:, None], 0.0)
    cost_im = np.maximum(MARGIN + S - diag[None, :], 0.0)
    cost_s[eye] = 0.0
    cost_im[eye] = 0.0
    return np.float32(cost_s.max(axis=1).sum() + cost_im.max(axis=0).sum())


def kernel(images, captions, img_lens, cap_lens):
    nc = _get_nc()
    in_maps = make_in_maps(images, captions, img_lens, cap_lens)
    res = run_bass_kernel_spmd(nc, in_maps, core_ids=list(range(NCORES)))
    se_list = [res.results[c]["se_out"] for c in range(NCORES)]
    return finish(se_list, img_lens)


# revision 12
# speedup vs baseline: 1.6071x; 1.6071x over previous
"""Trainium2 Bass kernel for the SCAN-style cross-attention contrastive loss.

Sharding: image axis across 8 cores (8 images/core), captions replicated.
Each core computes its 66x8 column block of per-(caption,image) exp-sum
scores; the host gathers columns and applies the scalar hinge-loss epilogue.

Math restructure (validated to ~1e-7 against the jax reference):
  - unnormalized softmax weights u = exp(9*A_norm + wbias); the softmax
    denominator cancels in sim = num/(n1*||wctx||).
  - num  = E^T (u .* Araw)          (per-column reduction via indicator matmul)
  - q    = E^T (u .* (G_blk @ u)) = ||wctx_unnorm||^2 via per-caption Gram
  - invalid image frames are zeroed on host => their columns give e = 1
    exactly; host subtracts the known defect (F - img_len) from each exp-sum.
"""
from contextlib import ExitStack

import numpy as np

import concourse.bacc as bacc
import concourse.tile as tile
from concourse import mybir
from concourse.bass_utils import run_bass_kernel_spmd

N, F, W, D = 64, 64, 40, 512
NCORES = 8
IPC = N // NCORES        # images per core = 8
IF = IPC * F             # 512 image-frame columns per core
GP = 3                   # captions per partition group
NCAP = 66                # 64 captions padded to a multiple of GP
NG = NCAP // GP          # 22 groups
GW = GP * W              # 120 partitions per group
DCH = D // 128           # 4 contraction chunks
SG = 6                   # groups per normalization super-block

f32 = mybir.dt.float32
f32r = mybir.dt.float32r
FT = mybir.ActivationFunctionType
ALU = mybir.AluOpType
AX = mybir.AxisListType

MARGIN = 0.2
LAMBDA_LSE = 6.0


def _r(ap):
    return ap.bitcast(f32r)


def _build_nc():
    nc = bacc.Bacc("TRN2", target_bir_lowering=False, debug=False)
    capT = nc.dram_tensor("capT", [128, NG, DCH, GW], f32, kind="ExternalInput").ap()
    imgT = nc.dram_tensor("imgT", [128, DCH, IF], f32, kind="ExternalInput").ap()
    wbias = nc.dram_tensor("wbias", [GW, NG], f32, kind="ExternalInput").ap()
    gmask = nc.dram_tensor("gmask", [GW, GW], f32, kind="ExternalInput").ap()
    emat = nc.dram_tensor("emat", [GW, GP], f32, kind="ExternalInput").ap()
    ones = nc.dram_tensor("ones", [128, 1], f32, kind="ExternalInput").ap()
    se_out = nc.dram_tensor("se_out", [NCAP, IPC], f32, kind="ExternalOutput").ap()

    with tile.TileContext(nc) as tc, ExitStack() as ctx:
        const = ctx.enter_context(tc.tile_pool(name="const", bufs=1))
        caps = ctx.enter_context(tc.tile_pool(name="caps", bufs=3))
        work = ctx.enter_context(tc.tile_pool(name="work", bufs=2))
        small = ctx.enter_context(tc.tile_pool(name="small", bufs=3))
        pa = ctx.enter_context(tc.tile_pool(name="pa", bufs=2, space="PSUM"))
        pg = ctx.enter_context(tc.tile_pool(name="pg", bufs=1, space="PSUM"))
        pb = ctx.enter_context(tc.tile_pool(name="pb", bufs=1, space="PSUM"))
        pq = ctx.enter_context(tc.tile_pool(name="pq", bufs=2, space="PSUM"))

        imgT_t = const.tile([128, DCH, IF], f32r)
        nc.sync.dma_start(out=imgT_t, in_=imgT.bitcast(f32r))
        wbias_t = const.tile([GW, NG], f32)
        nc.sync.dma_start(out=wbias_t, in_=wbias)
        gmask_t = const.tile([GW, GW], f32)
        nc.sync.dma_start(out=gmask_t, in_=gmask)
        emat_t = const.tile([GW, GP], f32r)
        nc.sync.dma_start(out=emat_t, in_=emat.bitcast(f32r))
        ones_col = const.tile([128, 1], f32r)
        nc.sync.dma_start(out=ones_col, in_=ones.bitcast(f32r))
        eps_col = const.tile([128, 1], f32)
        nc.vector.memset(eps_col, 1e-20)

        # n1sq[f] = ||image frame f||^2, replicated across partitions
        imgsq_t = const.tile([128, DCH, IF], f32r)
        nc.vector.tensor_mul(imgsq_t, imgT_t.bitcast(f32), imgT_t.bitcast(f32))
        n1p = pg.tile([1, IF], f32, tag="g")
        for c in range(DCH):
            nc.tensor.matmul(out=n1p, lhsT=ones_col, rhs=imgsq_t[:, c, :],
                             start=(c == 0), stop=(c == DCH - 1))
        n1row = const.tile([1, IF], f32)
        nc.scalar.copy(n1row, n1p)
        n1repl = const.tile([128, IF], f32)
        nc.gpsimd.partition_broadcast(n1repl, n1row[0:1, :])

        # q/num gathered into SBUF accumulators via ACT staging + small DMAs
        q_all = const.tile([NCAP, IF], f32)
        num_all = const.tile([NCAP, IF], f32)

        SBS = [(s, min(SG, NG - s)) for s in range(0, NG, SG)]
        for s0, sbn in SBS:
            r2sb = small.tile([GW, SG, IPC], f32, tag="r2sb")
            srsb = small.tile([GW, SG, IPC], f32, tag="srsb")
            rinvsb = small.tile([GW, SG, IPC], f32, tag="rinvsb")
            held = []
            for j in range(sbn):
                g = s0 + j
                capg = caps.tile([128, DCH, GW], f32r)
                nc.sync.dma_start(out=capg, in_=capT[:, g, :, :].bitcast(f32r))

                # Araw[w, if] = caption_word . image_frame
                araw_p = pa.tile([GW, IF], f32)
                for c in range(DCH):
                    nc.tensor.matmul(out=araw_p, lhsT=capg[:, c, :],
                                     rhs=imgT_t[:, c, :],
                                     start=(c == 0), stop=(c == DCH - 1))

                # per-caption Gram (block-diagonal after masking)
                gram_p = pg.tile([GW, GW], f32, tag="g")
                for c in range(DCH):
                    nc.tensor.matmul(out=gram_p, lhsT=capg[:, c, :],
                                     rhs=capg[:, c, :],
                                     start=(c == 0), stop=(c == DCH - 1))
                gblk_t = work.tile([GW, GW], f32r, bufs=SG + 1)
                nc.vector.tensor_mul(gblk_t, gram_p, gmask_t)

                araw_t = work.tile([GW, IF], f32, bufs=SG + 1)
                nc.scalar.copy(araw_t, araw_p)

                # leaky relu: L = max(0.1*A, A)
                L_t = work.tile([GW, IF], f32, bufs=SG + 1)
                nc.vector.scalar_tensor_tensor(out=L_t, in0=araw_t, scalar=0.1,
                                               in1=araw_t, op0=ALU.mult,
                                               op1=ALU.max)

                sq_t = work.tile([GW, IF], f32)
                nc.gpsimd.tensor_mul(sq_t, L_t, L_t)
                nc.vector.reduce_sum(r2sb[:, j, :],
                                     sq_t.rearrange("p (i f) -> p i f", f=F),
                                     axis=AX.X)
                held.append((araw_t, L_t, gblk_t))

            # one sqrt + one reciprocal per super-block
            nc.scalar.activation(srsb[:, 0:sbn, :], r2sb[:, 0:sbn, :], FT.Sqrt)
            nc.vector.reciprocal(rinvsb[:, 0:sbn, :], srsb[:, 0:sbn, :])

            for j in range(sbn):
                g = s0 + j
                araw_t, L_t, gblk_t = held[j]
                at_t = work.tile([GW, IF], f32)
                nc.vector.tensor_mul(at_t.rearrange("p (i f) -> p i f", f=F),
                                     L_t.rearrange("p (i f) -> p i f", f=F),
                                     rinvsb[:, j, :].to_broadcast([GW, IPC, F]))
                u_t = work.tile([GW, IF], f32r)
                nc.scalar.activation(u_t, at_t, FT.Exp, scale=9.0,
                                     bias=wbias_t[:, g:g + 1])

                b_p = pb.tile([GW, IF], f32)
                nc.tensor.matmul(out=b_p, lhsT=gblk_t, rhs=u_t,
                                 start=True, stop=True)

                p_t = work.tile([GW, IF], f32r)
                nc.vector.tensor_mul(p_t, u_t.bitcast(f32), b_p)
                q_t = work.tile([GW, IF], f32r)
                nc.gpsimd.tensor_mul(q_t, u_t.bitcast(f32), araw_t)

                qp = pq.tile([GP, IF], f32, tag="q")
                nc.tensor.matmul(out=qp, lhsT=emat_t, rhs=p_t,
                                 start=True, stop=True)
                nump = pq.tile([GP, IF], f32, tag="num")
                nc.tensor.matmul(out=nump, lhsT=emat_t, rhs=q_t,
                                 start=True, stop=True)
                qstg = small.tile([GP, IF], f32, tag="qstg")
                nc.scalar.copy(qstg, qp)
                nstg = small.tile([GP, IF], f32, tag="nstg")
                nc.scalar.copy(nstg, nump)
                nc.sync.dma_start(out=q_all[g * GP:(g + 1) * GP, :], in_=qstg)
                nc.sync.dma_start(out=num_all[g * GP:(g + 1) * GP, :], in_=nstg)

        # single kernel-wide epilogue:
        # sim = num / sqrt(q * n1sq), e = exp(6*sim), block-sum over frames
        qs_t = work.tile([NCAP, IF], f32, tag="qs_t")
        nc.vector.tensor_mul(qs_t, q_all, n1repl[0:NCAP, :])
        d_t = work.tile([NCAP, IF], f32, tag="d_t")
        nc.scalar.activation(d_t, qs_t, FT.Sqrt, bias=eps_col[0:NCAP, :])
        ri2_t = work.tile([NCAP, IF], f32, tag="ri2_t")
        nc.vector.reciprocal(ri2_t, d_t)
        sim_t = work.tile([NCAP, IF], f32, tag="sim_t")
        nc.vector.tensor_mul(sim_t, num_all, ri2_t)
        e_t = work.tile([NCAP, IF], f32, tag="e_t")
        nc.scalar.activation(e_t, sim_t, FT.Exp, scale=LAMBDA_LSE)
        seg_t = small.tile([NCAP, IPC], f32, tag="seg_t")
        nc.vector.reduce_sum(seg_t, e_t.rearrange("p (i f) -> p i f", f=F),
                             axis=AX.X)
        nc.sync.dma_start(out=se_out, in_=seg_t)

    nc.compile()
    return nc


_NC = None


def _get_nc():
    global _NC
    if _NC is None:
        _NC = _build_nc()
    return _NC


def make_in_maps(images, captions, img_lens, cap_lens):
    """Host-side input preparation (numpy only): shard/transpose/mask."""
    images = np.ascontiguousarray(np.asarray(images, np.float32))
    captions = np.ascontiguousarray(np.asarray(captions, np.float32))
    img_lens = np.asarray(img_lens).astype(np.int64)
    cap_lens = np.asarray(cap_lens).astype(np.int64)

    # captions padded to 66; dummies replicate caption 0 (avoids 0/0)
    caps_p = np.concatenate(
        [captions, np.broadcast_to(captions[0:1], (NCAP - N, W, D))], axis=0)
    # [128, NG, DCH, GW] with partition = d % 128, GW index = b*W + w
    capT_np = np.ascontiguousarray(
        caps_p.reshape(NG, GP, W, DCH, 128).transpose(4, 0, 3, 1, 2)
        .reshape(128, NG, DCH, GW))

    wbias_np = np.full((NCAP, W), np.float32(-1e30))
    for j in range(N):
        wbias_np[j, :cap_lens[j]] = 0.0
    wbias_np = np.ascontiguousarray(
        wbias_np.reshape(NG, GP * W).T.astype(np.float32))  # [GW, NG]

    gmask_np = np.zeros((GW, GW), np.float32)
    emat_np = np.zeros((GW, GP), np.float32)
    for b in range(GP):
        gmask_np[b * W:(b + 1) * W, b * W:(b + 1) * W] = 1.0
        emat_np[b * W:(b + 1) * W, b] = 1.0

    in_maps = []
    for core in range(NCORES):
        imgs = images[core * IPC:(core + 1) * IPC].copy()
        for i in range(IPC):
            imgs[i, img_lens[core * IPC + i]:] = 0.0
        Z = imgs.reshape(IF, D)
        imgT_np = np.ascontiguousarray(
            Z.reshape(IF, DCH, 128).transpose(2, 1, 0))  # [128, DCH, IF]
        in_maps.append({
            "capT": capT_np, "imgT": imgT_np, "wbias": wbias_np,
            "gmask": gmask_np, "emat": emat_np,
            "ones": np.ones((128, 1), np.float32),
        })
    return in_maps


def finish(se_list, img_lens):
    """Host epilogue: defect correction, log-sum-exp, hinge loss."""
    img_lens = np.asarray(img_lens).astype(np.int64)
    cols = []
    for core in range(NCORES):
        se = np.asarray(se_list[core], np.float32)[:N, :]         # (64, 8)
        defect = (F - img_lens[core * IPC:(core + 1) * IPC]).astype(np.float32)
        cols.append(np.log(se - defect[None, :]) / LAMBDA_LSE)
    S = np.concatenate(cols, axis=1).astype(np.float32)           # (caps, imgs)

    diag = np.diag(S)
    eye = np.eye(N, dtype=bool)
    cost_s = np.maximum(MARGIN + S - diag[:, None], 0.0)
    cost_im = np.maximum(MARGIN + S - diag[None, :], 0.0)
    cost_s[eye] = 0.0
    cost_im[eye] = 0.0
    return np.float32(cost_s.max(axis=1).sum() + cost_im.max(axis=0).sum())


def kernel(images, captions, img_lens, cap_lens):
    nc = _get_nc()
    in_maps = make_in_maps(images, captions, img_lens, cap_lens)
    res = run_bass_kernel_spmd(nc, in_maps, core_ids=list(range(NCORES)))
    se_list = [res.results[c]["se_out"] for c in range(NCORES)]
    return finish(se_list, img_lens)


# revision 13
# speedup vs baseline: 1.7132x; 1.0660x over previous
"""Trainium2 Bass kernel for the SCAN-style cross-attention contrastive loss.

Sharding: image axis across 8 cores (8 images/core), captions replicated.
Each core computes its 66x8 column block of per-(caption,image) exp-sum
scores; the host gathers columns and applies the scalar hinge-loss epilogue.

Math restructure (validated to ~1e-7 against the jax reference):
  - unnormalized softmax weights u = exp(9*A_norm + wbias); the softmax
    denominator cancels in sim = num/(n1*||wctx||).
  - num  = E^T (u .* Araw)          (per-column reduction via indicator matmul)
  - q    = E^T (u .* (G_blk @ u)) = ||wctx_unnorm||^2 via per-caption Gram
  - invalid image frames are zeroed on host => their columns give e = 1
    exactly; host subtracts the known defect (F - img_len) from each exp-sum.
"""
from contextlib import ExitStack

import numpy as np

import concourse.bacc as bacc
import concourse.tile as tile
from concourse import mybir
from concourse.bass_utils import run_bass_kernel_spmd

N, F, W, D = 64, 64, 40, 512
NCORES = 8
IPC = N // NCORES        # images per core = 8
IF = IPC * F             # 512 image-frame columns per core
GP = 3                   # captions per partition group
NCAP = 66                # 64 captions padded to a multiple of GP
NG = NCAP // GP          # 22 groups
GW = GP * W              # 120 partitions per group
DCH = D // 128           # 4 contraction chunks
SG = 4                   # groups per normalization super-block

f32 = mybir.dt.float32
f32r = mybir.dt.float32r
FT = mybir.ActivationFunctionType
ALU = mybir.AluOpType
AX = mybir.AxisListType

MARGIN = 0.2
LAMBDA_LSE = 6.0


def _r(ap):
    return ap.bitcast(f32r)


def _build_nc():
    nc = bacc.Bacc("TRN2", target_bir_lowering=False, debug=False)
    capT = nc.dram_tensor("capT", [128, NG, DCH, GW], f32, kind="ExternalInput").ap()
    imgT = nc.dram_tensor("imgT", [128, DCH, IF], f32, kind="ExternalInput").ap()
    wbias = nc.dram_tensor("wbias", [GW, NG], f32, kind="ExternalInput").ap()
    gmask = nc.dram_tensor("gmask", [GW, GW], f32, kind="ExternalInput").ap()
    emat = nc.dram_tensor("emat", [GW, GP], f32, kind="ExternalInput").ap()
    ones = nc.dram_tensor("ones", [128, 1], f32, kind="ExternalInput").ap()
    se_out = nc.dram_tensor("se_out", [NCAP, IPC], f32, kind="ExternalOutput").ap()

    with tile.TileContext(nc) as tc, ExitStack() as ctx:
        const = ctx.enter_context(tc.tile_pool(name="const", bufs=1))
        caps = ctx.enter_context(tc.tile_pool(name="caps", bufs=3))
        work = ctx.enter_context(tc.tile_pool(name="work", bufs=2))
        small = ctx.enter_context(tc.tile_pool(name="small", bufs=3))
        pa = ctx.enter_context(tc.tile_pool(name="pa", bufs=2, space="PSUM"))
        pg = ctx.enter_context(tc.tile_pool(name="pg", bufs=1, space="PSUM"))
        pb = ctx.enter_context(tc.tile_pool(name="pb", bufs=1, space="PSUM"))
        pq = ctx.enter_context(tc.tile_pool(name="pq", bufs=2, space="PSUM"))

        imgT_t = const.tile([128, DCH, IF], f32r)
        nc.sync.dma_start(out=imgT_t, in_=imgT.bitcast(f32r))
        wbias_t = const.tile([GW, NG], f32)
        nc.sync.dma_start(out=wbias_t, in_=wbias)
        gmask_t = const.tile([GW, GW], f32)
        nc.sync.dma_start(out=gmask_t, in_=gmask)
        emat_t = const.tile([GW, GP], f32r)
        nc.sync.dma_start(out=emat_t, in_=emat.bitcast(f32r))
        ones_col = const.tile([128, 1], f32r)
        nc.sync.dma_start(out=ones_col, in_=ones.bitcast(f32r))
        eps_col = const.tile([128, 1], f32)
        nc.vector.memset(eps_col, 1e-20)

        # n1sq[f] = ||image frame f||^2, replicated across partitions
        imgsq_t = const.tile([128, DCH, IF], f32r)
        nc.vector.tensor_mul(imgsq_t, imgT_t.bitcast(f32), imgT_t.bitcast(f32))
        n1p = pg.tile([1, IF], f32, tag="g")
        for c in range(DCH):
            nc.tensor.matmul(out=n1p, lhsT=ones_col, rhs=imgsq_t[:, c, :],
                             start=(c == 0), stop=(c == DCH - 1))
        n1row = const.tile([1, IF], f32)
        nc.scalar.copy(n1row, n1p)
        n1repl = const.tile([128, IF], f32)
        nc.gpsimd.partition_broadcast(n1repl, n1row[0:1, :])

        # q|num gathered into one SBUF accumulator via ACT staging + DMAs
        qn_all = const.tile([NCAP, 2, IF], f32)

        SBS = [(s, min(SG, NG - s)) for s in range(0, NG, SG)]
        for s0, sbn in SBS:
            r2sb = small.tile([GW, SG, IPC], f32, tag="r2sb")
            srsb = small.tile([GW, SG, IPC], f32, tag="srsb")
            rinvsb = small.tile([GW, SG, IPC], f32, tag="rinvsb")
            held = []
            for j in range(sbn):
                g = s0 + j
                capg = caps.tile([128, DCH, GW], f32r, bufs=4)
                nc.sync.dma_start(out=capg, in_=capT[:, g, :, :].bitcast(f32r))

                # Araw[w, if] = caption_word . image_frame
                araw_p = pa.tile([GW, IF], f32)
                for c in range(DCH):
                    nc.tensor.matmul(out=araw_p, lhsT=capg[:, c, :],
                                     rhs=imgT_t[:, c, :],
                                     start=(c == 0), stop=(c == DCH - 1))

                # per-caption Gram (block-diagonal after masking)
                gram_p = pg.tile([GW, GW], f32, tag="g")
                for c in range(DCH):
                    nc.tensor.matmul(out=gram_p, lhsT=capg[:, c, :],
                                     rhs=capg[:, c, :],
                                     start=(c == 0), stop=(c == DCH - 1))
                gblk_t = work.tile([GW, GW], f32r, bufs=SG + 1)
                nc.vector.tensor_mul(gblk_t, gram_p, gmask_t)

                araw_t = work.tile([GW, IF], f32, bufs=SG + 1)
                nc.scalar.copy(araw_t, araw_p)

                # leaky relu: L = max(0.1*A, A)
                L_t = work.tile([GW, IF], f32, bufs=SG + 1)
                nc.vector.scalar_tensor_tensor(out=L_t, in0=araw_t, scalar=0.1,
                                               in1=araw_t, op0=ALU.mult,
                                               op1=ALU.max)

                sq_t = work.tile([GW, IF], f32, bufs=3)
                nc.gpsimd.tensor_mul(sq_t, L_t, L_t)
                nc.vector.reduce_sum(r2sb[:, j, :],
                                     sq_t.rearrange("p (i f) -> p i f", f=F),
                                     axis=AX.X)
                held.append((araw_t, L_t, gblk_t))

            # one sqrt + one reciprocal per super-block
            nc.scalar.activation(srsb[:, 0:sbn, :], r2sb[:, 0:sbn, :], FT.Sqrt)
            nc.vector.reciprocal(rinvsb[:, 0:sbn, :], srsb[:, 0:sbn, :])

            for j in range(sbn):
                g = s0 + j
                araw_t, L_t, gblk_t = held[j]
                at_t = work.tile([GW, IF], f32, bufs=3)
                nc.vector.tensor_mul(at_t.rearrange("p (i f) -> p i f", f=F),
                                     L_t.rearrange("p (i f) -> p i f", f=F),
                                     rinvsb[:, j, :].to_broadcast([GW, IPC, F]))
                u_t = work.tile([GW, IF], f32r, bufs=3)
                nc.scalar.activation(u_t, at_t, FT.Exp, scale=9.0,
                                     bias=wbias_t[:, g:g + 1])

                b_p = pb.tile([GW, IF], f32)
                nc.tensor.matmul(out=b_p, lhsT=gblk_t, rhs=u_t,
                                 start=True, stop=True)

                p_t = work.tile([GW, IF], f32r, bufs=3)
                nc.vector.tensor_mul(p_t, u_t.bitcast(f32), b_p)
                q_t = work.tile([GW, IF], f32r, bufs=3)
                nc.gpsimd.tensor_mul(q_t, u_t.bitcast(f32), araw_t)

                qnp = pq.tile([GP, 2, IF], f32, tag="qn")
                nc.tensor.matmul(out=qnp[:, 0, :], lhsT=emat_t, rhs=p_t,
                                 start=True, stop=True)
                nc.tensor.matmul(out=qnp[:, 1, :], lhsT=emat_t, rhs=q_t,
                                 start=True, stop=True)
                qnstg = small.tile([GP, 2, IF], f32, tag="qnstg")
                nc.scalar.copy(qnstg, qnp)
                nc.sync.dma_start(out=qn_all[g * GP:(g + 1) * GP, :, :],
                                  in_=qnstg)

        # single kernel-wide epilogue:
        # sim = num / sqrt(q * n1sq), e = exp(6*sim), block-sum over frames
        qs_t = work.tile([NCAP, IF], f32, tag="qs_t")
        nc.vector.tensor_mul(qs_t, qn_all[:, 0, :], n1repl[0:NCAP, :])
        d_t = work.tile([NCAP, IF], f32, tag="d_t")
        nc.scalar.activation(d_t, qs_t, FT.Sqrt, bias=eps_col[0:NCAP, :])
        ri2_t = work.tile([NCAP, IF], f32, tag="ri2_t")
        nc.vector.reciprocal(ri2_t, d_t)
        sim_t = work.tile([NCAP, IF], f32, tag="sim_t")
        nc.vector.tensor_mul(sim_t, qn_all[:, 1, :], ri2_t)
        e_t = work.tile([NCAP, IF], f32, tag="e_t")
        nc.scalar.activation(e_t, sim_t, FT.Exp, scale=LAMBDA_LSE)
        seg_t = small.tile([NCAP, IPC], f32, tag="seg_t")
        nc.vector.reduce_sum(seg_t, e_t.rearrange("p (i f) -> p i f", f=F),
                             axis=AX.X)
        nc.sync.dma_start(out=se_out, in_=seg_t)

    nc.compile()
    return nc


_NC = None


def _get_nc():
    global _NC
    if _NC is None:
        _NC = _build_nc()
    return _NC


def make_in_maps(images, captions, img_lens, cap_lens):
    """Host-side input preparation (numpy only): shard/transpose/mask."""
    images = np.ascontiguousarray(np.asarray(images, np.float32))
    captions = np.ascontiguousarray(np.asarray(captions, np.float32))
    img_lens = np.asarray(img_lens).astype(np.int64)
    cap_lens = np.asarray(cap_lens).astype(np.int64)

    # captions padded to 66; dummies replicate caption 0 (avoids 0/0)
    caps_p = np.concatenate(
        [captions, np.broadcast_to(captions[0:1], (NCAP - N, W, D))], axis=0)
    # [128, NG, DCH, GW] with partition = d % 128, GW index = b*W + w
    capT_np = np.ascontiguousarray(
        caps_p.reshape(NG, GP, W, DCH, 128).transpose(4, 0, 3, 1, 2)
        .reshape(128, NG, DCH, GW))

    wbias_np = np.full((NCAP, W), np.float32(-1e30))
    for j in range(N):
        wbias_np[j, :cap_lens[j]] = 0.0
    wbias_np = np.ascontiguousarray(
        wbias_np.reshape(NG, GP * W).T.astype(np.float32))  # [GW, NG]

    gmask_np = np.zeros((GW, GW), np.float32)
    emat_np = np.zeros((GW, GP), np.float32)
    for b in range(GP):
        gmask_np[b * W:(b + 1) * W, b * W:(b + 1) * W] = 1.0
        emat_np[b * W:(b + 1) * W, b] = 1.0

    in_maps = []
    for core in range(NCORES):
        imgs = images[core * IPC:(core + 1) * IPC].copy()
        for i in range(IPC):
            imgs[i, img_lens[core * IPC + i]:] = 0.0
        Z = imgs.reshape(IF, D)
        imgT_np = np.ascontiguousarray(
            Z.reshape(IF, DCH, 128).transpose(2, 1, 0))  # [128, DCH, IF]
        in_maps.append({
            "capT": capT_np, "imgT": imgT_np, "wbias": wbias_np,
            "gmask": gmask_np, "emat": emat_np,
            "ones": np.ones((128, 1), np.float32),
        })
    return in_maps


def finish(se_list, img_lens):
    """Host epilogue: defect correction, log-sum-exp, hinge loss."""
    img_lens = np.asarray(img_lens).astype(np.int64)
    cols = []
    for core in range(NCORES):
        se = np.asarray(se_list[core], np.float32)[:N, :]         # (64, 8)
        defect = (F - img_lens[core * IPC:(core + 1) * IPC]).astype(np.float32)
        cols.append(np.log(se - defect[None, :]) / LAMBDA_LSE)
    S = np.concatenate(cols, axis=1).astype(np.float32)           # (caps, imgs)

    diag = np.diag(S)
    eye = np.eye(N, dtype=bool)
    cost_s = np.maximum(MARGIN + S - diag[:, None], 0.0)
    cost_im = np.maximum(MARGIN + S - diag[None, :], 0.0)
    cost_s[eye] = 0.0
    cost_im[eye] = 0.0
    return np.float32(cost_s.max(axis=1).sum() + cost_im.max(axis=0).sum())


def kernel(images, captions, img_lens, cap_lens):
    nc = _get_nc()
    in_maps = make_in_maps(images, captions, img_lens, cap_lens)
    res = run_bass_kernel_spmd(nc, in_maps, core_ids=list(range(NCORES)))
    se_list = [res.results[c]["se_out"] for c in range(NCORES)]
    return finish(se_list, img_lens)
